# revision 5
# baseline (speedup 1.0000x reference)
"""Trainium2 Bass kernel for nn_MCUDetectionLoss.

Strategy (data-parallel over batch, 8 cores, B=16 -> 2 images/core):

The loss touches (a) the objectness channel cls_p[:, 0] in full and (b) 64+64
gathered cells per core (obj/cls/reg values at target cells).  The host slices
each core's two images, gathers the 128 target rows (cheap fancy indexing),
and ships two small tensors per core:
  - gath [128, 74]   per-target row: aux cols and the 66-col activation
                     block [obj, cls63, -r0, -r1] plus -clip(r2..r3) so a
                     single Exp/Ln/Exp chain yields softplus, 1-p, sigmoid
                     and exp(clip) in one pass
  - objd [128, 320]  objectness maps (scale3 flat 32768 = cols 0:256,
                     scale4 flat 8192 = cols 256:320)

Device program per core: two input DMAs, a 6-op scalar ACT chain (exp/ln
softplus over the gathered block and the full obj map, one activation-table
set), and a 13-op DVE chain computing focal and smooth-L1 partials with
free-axis accumulation.  Output is a [128, 7] per-partition stats tile; the
host reduces the 8x128 rows in float64.

Identities used (bce = BCEWithLogits):
  bce(x, 0) = softplus(x);  bce(x, 1) = softplus(x) - x
  focal (1-pt)^2 = (p-y)^2; 1-p = exp(-softplus(x)); sigmoid = exp(-softplus(-x))
  smooth_l1(d) = 0.5 d^2 - 0.5 relu(d-1)^2 - 0.5 relu(-d-1)^2
  sum softplus(obj)*bg = sum_all softplus - sum_targets softplus(obj_t)/count_t
The device computes focal with the y=0 form for ALL classes; the host adds an
exact f64 per-target correction for the single true class.  Duplicate-cell
counts and unique-cell counts are computed on host.
"""

import sys

for _p in ("/opt/trn_rl_repo", "/root/.axon_site/_ro/trn_rl_repo"):
    if _p not in sys.path:
        sys.path.append(_p)

import numpy as np

import concourse.bass as bass
from concourse import mybir
from concourse.bass_utils import run_bass_kernel_spmd

AF = mybir.ActivationFunctionType
ALU = mybir.AluOpType
AX = mybir.AxisListType
F32 = mybir.dt.float32

ALPHA = 0.25
BBOX_W, OBJ_W, CLS_W = 2.0, 1.0, 0.5

M = 8          # cores
B, T, NC_CLS = 16, 32, 63
H3 = W3 = 128
H4 = W4 = 64
BL = B // M    # images per core
N3 = BL * H3 * W3   # 32768 scale3 cells per core
N4 = BL * H4 * W4   # 8192 scale4 cells per core
C3 = N3 // 128      # 256 obj cols of scale3
OBJW = (N3 + N4) // 128  # 320
NT = 2 * BL * T     # 128 targets per core (rows 0:64 scale3, 64:128 scale4)

# gath column layout
G_OBJ = 0            # obj_g (for s1)
G_RCNT = 1           # 1/count
G_K4 = 2             # [g2x, g2y, -tw, -th]
G_GA = 6             # [obj, cls63, -r0, -r1] (66) -> overwritten by softplus
G_NCL = 72           # -clip(r2), -clip(r3)
G_W = 74

_NC_CACHE = None


def _build_bass():
    nc = bass.Bass("TRN2", target_bir_lowering=False, debug=False, num_devices=M)
    gath = nc.declare_dram_parameter("gath", [NT, G_W], F32, isOutput=False)
    objd = nc.declare_dram_parameter("objd", [128, OBJW], F32, isOutput=False)
    outd = nc.declare_dram_parameter("outp", [NT, 7], F32, isOutput=True)

    from contextlib import ExitStack
    with ExitStack() as st:
        def sb(name, shape, dt=F32):
            return st.enter_context(nc.sbuf_tensor(name, shape, dt))

        GT = sb("GT", [NT, G_W])
        E66 = sb("E66", [NT, 66])
        RX = sb("RX", [NT, 68])
        OBJ = sb("OBJ", [128, OBJW])
        EO = sb("EO", [128, OBJW])
        SP = sb("SP", [128, OBJW])
        U0 = sb("U0", [NT, NC_CLS])
        Q0 = sb("Q0", [NT, NC_CLS])
        F0 = sb("F0", [NT, NC_CLS])
        AC = sb("AC", [NT, 4])
        DT = sb("DT", [NT, 4])
        SQ = sb("SQ", [NT, 4])
        MM = sb("MM", [NT, 8])
        MS = sb("MS", [NT, 8])
        ST = sb("ST", [NT, 7])
        WT = sb("WT", [128, 1])

        g_sem = st.enter_context(nc.semaphore("g_sem"))
        o_sem = st.enter_context(nc.semaphore("o_sem"))
        a_sem = st.enter_context(nc.semaphore("a_sem"))
        d_sem = st.enter_context(nc.semaphore("d_sem"))
        st_sem = st.enter_context(nc.semaphore("st_sem"))
        block = st.enter_context(nc.Block(no_gpsimd_drain=True))

        # scl (softplus of GA) overwrites GT[:, G_GA:G_GA+66] in place
        scl0 = GT[:, G_GA:G_GA + 1]          # softplus(obj_g)
        sclx = GT[:, G_GA + 1:G_GA + 64]     # softplus(cls)

        @block.sync
        def _(sync):
            sync.dma_start(out=GT[:], in_=gath[:]).then_inc(g_sem, 16)
            sync.dma_start(out=OBJ[:], in_=objd[:]).then_inc(o_sem, 16)
            sync.wait_ge(a_sem, 6)
            sync.wait_ge(d_sem, 13)
            sync.dma_start(out=outd[:], in_=ST[:]).then_inc(st_sem, 16)

        @block.gpsimd
        def _(gpsimd):
            pass

        @block.tensor
        def _(tensor):
            pass

        @block.scalar
        def _(scalar):
            act = nc.scalar
            # warmup: load the exp/ln ACT table before data arrives
            act.activation(out=WT[:], in_=WT[:],
                           func=AF.Exp).then_inc(a_sem, 1)                  # 1
            scalar.wait_ge(g_sem, 16)
            act.activation(out=E66[:], in_=GT[:, G_GA:G_GA + 66],
                           func=AF.Exp).then_inc(a_sem, 1)                  # 2
            act.activation(out=GT[:, G_GA:G_GA + 66], in_=E66[:],
                           func=AF.Ln, bias=1.0).then_inc(a_sem, 1)         # 3
            act.activation(out=RX[:], in_=GT[:, G_GA:G_GA + 68],
                           func=AF.Exp, scale=-1.0).then_inc(a_sem, 1)      # 4
            scalar.wait_ge(o_sem, 16)
            act.activation(out=EO[:], in_=OBJ[:],
                           func=AF.Exp).then_inc(a_sem, 1)                  # 5
            act.activation(out=SP[:], in_=EO[:], func=AF.Ln, bias=1.0,
                           accum_out=ST[:, 5:6]).then_inc(a_sem, 1)         # 6

        @block.vector
        def _(vector):
            vec = nc.vector
            vector.wait_ge(a_sem, 3)
            vec.tensor_tensor(out=ST[:, 2:3], in0=scl0, in1=GT[:, 0:1],
                              op=ALU.subtract).then_inc(d_sem, 1)           # 1
            vec.tensor_tensor(out=ST[:, 4:5], in0=scl0, in1=GT[:, 1:2],
                              op=ALU.mult).then_inc(d_sem, 1)               # 2
            vector.wait_ge(a_sem, 4)
            vec.tensor_scalar_add(out=U0[:], in0=RX[:, 1:64],
                                  scalar1=-1.0).then_inc(d_sem, 1)          # 3
            vec.tensor_tensor(out=AC[:], in0=RX[:, 64:68],
                              in1=GT[:, G_K4:G_K4 + 4],
                              op=ALU.add).then_inc(d_sem, 1)                # 4
            nc.vector.drain()
            vec.tensor_tensor(out=Q0[:], in0=U0[:], in1=U0[:],
                              op=ALU.mult).then_inc(d_sem, 1)               # 5
            vec.scalar_tensor_tensor(out=DT[:, 0:2], in0=AC[:, 2:4],
                                     scalar=-0.5, in1=AC[:, 0:2],
                                     op0=ALU.mult,
                                     op1=ALU.add).then_inc(d_sem, 1)        # 6
            vec.scalar_tensor_tensor(out=DT[:, 2:4], in0=AC[:, 2:4],
                                     scalar=0.5, in1=AC[:, 0:2],
                                     op0=ALU.mult,
                                     op1=ALU.add).then_inc(d_sem, 1)        # 7
            nc.vector.drain()
            vec.scalar_tensor_tensor(out=F0[:], in0=Q0[:], scalar=ALPHA / 63.0,
                                     in1=sclx, op0=ALU.mult, op1=ALU.mult,
                                     accum_out=ST[:, 3:4]).then_inc(d_sem, 1)  # 8
            vec.scalar_tensor_tensor(out=SQ[:], in0=DT[:], scalar=1.0,
                                     in1=DT[:], op0=ALU.mult, op1=ALU.mult,
                                     accum_out=ST[:, 0:1]).then_inc(d_sem, 1)  # 9
            vec.tensor_scalar(out=MM[:, 0:4], in0=DT[:], scalar1=1.0,
                              scalar2=-1.0, op0=ALU.max,
                              op1=ALU.add).then_inc(d_sem, 1)               # 10
            vec.tensor_scalar(out=MM[:, 4:8], in0=DT[:], scalar1=-1.0,
                              scalar2=1.0, op0=ALU.min,
                              op1=ALU.add).then_inc(d_sem, 1)               # 11
            nc.vector.drain()
            vec.scalar_tensor_tensor(out=MS[:], in0=MM[:], scalar=1.0,
                                     in1=MM[:], op0=ALU.mult, op1=ALU.mult,
                                     accum_out=ST[:, 1:2]).then_inc(d_sem, 1)  # 12
            vector.wait_ge(a_sem, 6)
            vec.reduce_sum(out=ST[:, 6:7], in_=SP[:, C3:OBJW],
                           axis=AX.X).then_inc(d_sem, 1)                    # 13

    return nc


def _get_bass():
    global _NC_CACHE
    if _NC_CACHE is None:
        _NC_CACHE = _build_bass()
    return _NC_CACHE


def _prep_core_inputs(cls_p3, reg_p3, cls_p4, reg_p4, t3, t4):
    """Slice + gather full inputs into the per-core input maps.

    Also returns the f64 focal correction sum (device computes the y=0 focal
    form for every class; the true class needs the y=1 form)."""
    f = np.float32
    in_maps = []
    fcorr = 0.0
    for c in range(M):
        sl = slice(c * BL, (c + 1) * BL)
        gath = np.zeros((NT, G_W), f)
        objs = []
        for s, (cp, rp, lt, hh, ww) in enumerate([
                (cls_p3[sl], reg_p3[sl], t3[sl], H3, W3),
                (cls_p4[sl], reg_p4[sl], t4[sl], H4, W4)]):
            rows = slice(s * BL * T, (s + 1) * BL * T)
            tx = (lt[..., 1] * ww).astype(f)
            ty = (lt[..., 2] * hh).astype(f)
            tw = (lt[..., 3] * ww).astype(f)
            th = (lt[..., 4] * hh).astype(f)
            gx = np.clip(tx, 0, ww - 1).astype(np.int32)
            gy = np.clip(ty, 0, hh - 1).astype(np.int32)
            cid = lt[..., 0].astype(np.int32)
            bb = np.arange(BL)[:, None]

            cls_g = cp[bb, :, gy, gx].astype(f)     # [BL,T,64]
            reg_g = rp[bb, :, gy, gx].astype(f)     # [BL,T,4]
            obj_g = cls_g[..., 0]

            # duplicate-cell counts per (image, cell)
            key = (bb * (hh * ww) + gy * ww + gx).reshape(-1)
            _, inv, cnt = np.unique(key, return_inverse=True,
                                    return_counts=True)
            rcnt = (1.0 / cnt[inv]).astype(f).reshape(BL, T)

            # f64 focal correction for the true class (y=1 vs y=0 form)
            xs = np.take_along_axis(
                cls_g[..., 1:].astype(np.float64), cid[..., None], axis=-1
            )[..., 0]
            sp = np.logaddexp(0.0, xs)
            rx = np.exp(-sp)                     # 1 - sigmoid(x)
            fcorr += (ALPHA / NC_CLS) * float(
                (rx * rx * (sp - xs) - (rx - 1.0) ** 2 * sp).sum())

            g = np.zeros((BL * T, G_W), f)
            g[:, G_OBJ] = obj_g.reshape(-1)
            g[:, G_RCNT] = rcnt.reshape(-1)
            g[:, G_K4 + 0] = (gx - tx).reshape(-1)
            g[:, G_K4 + 1] = (gy - ty).reshape(-1)
            g[:, G_K4 + 2] = (-tw).reshape(-1)
            g[:, G_K4 + 3] = (-th).reshape(-1)
            g[:, G_GA] = obj_g.reshape(-1)
            g[:, G_GA + 1:G_GA + 64] = cls_g[..., 1:].reshape(-1, 63)
            g[:, G_GA + 64:G_GA + 66] = (-reg_g[..., 0:2]).reshape(-1, 2)
            g[:, G_NCL:G_NCL + 2] = (
                -np.clip(reg_g[..., 2:4], -4.0, 4.0)).reshape(-1, 2)
            gath[rows] = g
            objs.append(cp[:, 0].reshape(-1))

        obj = np.concatenate(objs).reshape(128, OBJW)
        in_maps.append({
            "gath": np.ascontiguousarray(gath),
            "objd": np.ascontiguousarray(obj, f),
        })
    return in_maps, fcorr


def _uniq_cells(t, hh, ww):
    tx = t[..., 1] * ww
    ty = t[..., 2] * hh
    gx = np.clip(tx, 0, ww - 1).astype(np.int64)
    gy = np.clip(ty, 0, hh - 1).astype(np.int64)
    bb = np.broadcast_to(np.arange(t.shape[0])[:, None], gx.shape)
    key = bb * (hh * ww) + gy * ww + gx
    return len(np.unique(key))


def _combine(parts, uniq3, uniq4, fcorr):
    """parts: [M, 128, 7] per-core stats -> scalar loss (float64 combine)."""
    P = np.asarray(parts, np.float64)
    # cols: 0 sum dt^2, 1 sum m^2, 2 obj-pos bce, 3 focal(y=0 form),
    #       4 spo*rcnt, 5 softplus sum (all cells), 6 softplus sum (scale4)
    lb_total = (P[:, :, 0].sum() - P[:, :, 1].sum()) / 8.0
    lo_pos = P[:, :, 2].sum()
    lc_total = P[:, :, 3].sum() + fcorr
    corr3 = P[:, 0:64, 4].sum()
    corr4 = P[:, 64:128, 4].sum()
    s_tot = P[:, :, 5].sum()
    s4 = P[:, :, 6].sum()
    s3 = s_tot - s4

    bg3 = (s3 - corr3) / max(B * H3 * W3 - uniq3, 1.0)
    bg4 = (s4 - corr4) / max(B * H4 * W4 - uniq4, 1.0)
    n = 2 * B * T
    lb = lb_total / n
    lc = lc_total / n
    lo = (lo_pos + 0.05 * (bg3 + bg4)) / max(n, 1)
    return np.float32(BBOX_W * lb + OBJ_W * lo + CLS_W * lc)


def kernel(cls_p3, reg_p3, cls_p4, reg_p4, t3, t4, _trace=False):
    cls_p3, reg_p3 = np.asarray(cls_p3), np.asarray(reg_p3)
    cls_p4, reg_p4 = np.asarray(cls_p4), np.asarray(reg_p4)
    t3, t4 = np.asarray(t3), np.asarray(t4)
    in_maps, fcorr = _prep_core_inputs(cls_p3, reg_p3, cls_p4, reg_p4, t3, t4)
    uniq3 = _uniq_cells(t3, H3, W3)
    uniq4 = _uniq_cells(t4, H4, W4)
    nc = _get_bass()
    res = run_bass_kernel_spmd(nc, in_maps, core_ids=list(range(M)),
                               trace=_trace)
    parts = np.stack([r["outp"] for r in res.results])
    out = _combine(parts, uniq3, uniq4, fcorr)
    if _trace:
        return out, res
    return out


if __name__ == "__main__":
    rng = np.random.default_rng(0)
    inputs = {
        "cls_p3": rng.standard_normal((B, 64, H3, W3)).astype(np.float32),
        "reg_p3": rng.standard_normal((B, 4, H3, W3)).astype(np.float32),
        "cls_p4": rng.standard_normal((B, 64, H4, W4)).astype(np.float32),
        "reg_p4": rng.standard_normal((B, 4, H4, W4)).astype(np.float32),
        "t3": rng.random((B, T, 5)).astype(np.float32),
        "t4": rng.random((B, T, 5)).astype(np.float32),
    }
    print(kernel(**inputs))
